# revision 7
# baseline (speedup 1.0000x reference)
"""DialogueEIN fused kernel for 8 TRN2 NeuronCores (data-parallel over batch).

Self-contained: hardcodes shapes for the nn_DialogueEIN problem
  x[64,256,512], T=256, H=512, NH=8 heads, E=7 emotion slots, window 5.

Strategy (per core, 8 batches):
  - All activations live in "transposed" space [H, T] so attention scores are
    computed directly as S_T[k, j] (keys on partitions, queries on free dim):
    qT/kT come straight out of the projection matmuls, softmax needs no
    transposes anywhere.
  - Softmax without max-subtraction: scores are O(1) and the additive mask
    bias is -50 instead of -1e4 (identical through softmax: fully-masked rows
    reduce to the reference's plain softmax, partially-masked rows leave
    masked weights at ~e^-48 relative -- below fp32 noise).
  - The per-row denominator d_j = sum_k exp(s) falls out of the PV matmul via
    a ones-column appended to V (lhsT [Tk, 65]); the reciprocal row is
    partition-broadcast via a DRAM bounce and folded into the PSUM eviction.
  - Mask biases for global/intra/inter are rank<=3 outer products accumulated
    into the score PSUM by tiny extra matmuls (host ships factor vectors).
    The local sliding-window branch uses a multiplicative post-exp mask
    built per batch from a constant band matrix (2 rank-1 matmuls + 2 DVE ops).
  - Host folds: b_Wo[i] @ W1_i (kills the concat+W1 matmul), ln2 gamma/beta
    into W2, t_bv/b_bv into downstream biases, 1/sqrt(dh) into Wq.
  - LayerNorm over the partition axis: ones-column matmuls for mean/E[x^2],
    DRAM-bounce broadcast of rstd / (-mu*rstd) rows, per-partition gamma/beta.
  - All matmuls run as float32r (full-rate fp32 mode on the PE).
"""

import numpy as np

import concourse.bass as bass
import concourse.mybir as mybir
import concourse.tile as tile
from concourse import bacc
from concourse.bass_utils import run_bass_kernel_spmd

F32 = mybir.dt.float32
F32R = mybir.dt.float32r
ALU = mybir.AluOpType
ACTF = mybir.ActivationFunctionType

B, T, H, NH, E = 64, 256, 512, 8, 7
DH = H // NH
NCORES = 8
BL = B // NCORES
NEG = -50.0
KT = H // 128  # 4
MT = T // 128  # 2
EPS = 1e-12

_CACHE = {}


def _dbc_ap(dram_ap, nparts, nfree):
    return bass.AP(tensor=dram_ap.tensor, offset=dram_ap.offset,
                   ap=[[0, nparts], [1, nfree]])


def _build(apply_g1b1, apply_c2):
    nc = bacc.Bacc("TRN2", target_bir_lowering=False, debug=False,
                   enable_asserts=False)

    def din(name, shape, dt=F32R):
        return nc.dram_tensor(name, list(shape), dt, kind="ExternalInput").ap()

    xT = din("xT", (BL, H, T))
    flq = din("flq", (BL, 3, T))
    frintra = din("frintra", (BL, 3, T))
    frinter = din("frinter", (BL, 3, T))
    frg = din("frg", (BL, 1, T))
    locv = din("locv", (BL, 3, T))
    bandd = din("bandd", (T, T), F32)
    kTemo = din("kTemo", (H, E))
    vemoaug = din("vemoaug", (E, NH * 65))
    tWq = din("tWq", (H, H))
    tWo = din("tWo", (H, H))
    bWq = din("bWq", (4, H, H))
    bWk = din("bWk", (4, H, H))
    bWv = din("bWv", (4, H, H))
    What = din("What", (4, H, H))
    W2p = din("W2p", (H, H))
    onesd = din("onesd", (T,))
    tbq = din("tbq", (H,), F32)
    tbo = din("tbo", (H,), F32)
    g1 = din("g1", (H,), F32)
    b1v = din("b1v", (H,), F32)
    bbq = din("bbq", (4, H), F32)
    bbk = din("bbk", (4, H), F32)
    bhat = din("bhat", (H,), F32)
    c2row = din("c2row", (H,), F32)
    out = nc.dram_tensor("out", [BL, T, H], F32, kind="ExternalOutput").ap()

    with tile.TileContext(nc) as tc:
        cst = tc.alloc_tile_pool(name="cst", bufs=1)
        per = tc.alloc_tile_pool(name="per", bufs=1)
        wts = tc.alloc_tile_pool(name="wts", bufs=1)
        act = tc.alloc_tile_pool(name="act", bufs=1)
        pmm = tc.alloc_tile_pool(name="pmm", bufs=3, space="PSUM")
        psc = tc.alloc_tile_pool(name="psc", bufs=3, space="PSUM")
        pcx = tc.alloc_tile_pool(name="pcx", bufs=2, space="PSUM")
        drp = tc.alloc_tile_pool(name="drp", bufs=8, space="DRAM")

        # ---- constants ----
        ones128 = cst.tile([128, 1], F32R, name="ones128")
        nc.sync.dma_start(out=ones128, in_=onesd[0:128])
        kTe = []
        for k in range(KT):
            t = cst.tile([128, E], F32R, name=f"kTemo{k}")
            nc.sync.dma_start(out=t, in_=kTemo[k * 128:(k + 1) * 128, :])
            kTe.append(t)
        eps_t = cst.tile([1, 1], F32, name="eps_t")
        nc.vector.memset(eps_t, EPS)
        onr = cst.tile([1, T], F32R, name="onr")
        nc.sync.dma_start(out=onr, in_=onesd)
        vea = cst.tile([E, NH * 65], F32R, name="vemoaug")
        nc.sync.dma_start(out=vea, in_=vemoaug)
        bandt = []
        for m in range(MT):
            t = cst.tile([128, T], F32, name=f"band{m}")
            nc.sync.dma_start(out=t, in_=bandd[m * 128:(m + 1) * 128, :])
            bandt.append(t)

        def vec_tiles(v, nm):
            ts = []
            for k in range(KT):
                t = cst.tile([128, 1], F32, name=f"{nm}{k}")
                nc.sync.dma_start(out=t, in_=v[k * 128:(k + 1) * 128])
                ts.append(t)
            return ts

        tbq_t = vec_tiles(tbq, "tbq")
        tbo_t = vec_tiles(tbo, "tbo")
        bhat_t = vec_tiles(bhat, "bhat")
        g1_t = vec_tiles(g1, "g1") if apply_g1b1 else None
        b1_t = vec_tiles(b1v, "b1v") if apply_g1b1 else None
        bbq_t = [vec_tiles(bbq[i], f"bbq{i}") for i in range(4)]
        bbk_t = [vec_tiles(bbk[i], f"bbk{i}") for i in range(4)]
        c2n = None
        if apply_c2:
            c2n = cst.tile([128, H], F32, name="c2n")
            nc.sync.dma_start(out=c2n, in_=_dbc_ap(c2row, 128, H))

        # persistent per-batch state
        htT = [[per.tile([128, T], F32R, name=f"htT_{b}_{k}") for k in range(KT)]
               for b in range(BL)]
        h2sb = [[per.tile([128, T], F32R, name=f"h2sb_{b}_{k}")
                 for k in range(KT)] for b in range(BL)]

        def proj_T(wtiles, rhs_tiles, bias_tiles, tag):
            """[H,T] = W.T @ rhs, +bias per-partition on eviction."""
            res = []
            for mo in range(KT):
                ps = pmm.tile([128, T], F32, tag="mm", bufs=3)
                for ki in range(KT):
                    nc.tensor.matmul(
                        ps, wtiles[ki][:, mo * 128:(mo + 1) * 128],
                        rhs_tiles[ki], start=(ki == 0), stop=(ki == KT - 1))
                s = act.tile([128, T], F32R, tag=tag, bufs=8)
                nc.vector.tensor_scalar(s, ps, bias_tiles[mo], None, op0=ALU.add)
                res.append(s)
            return res

        def softmax_pv(h, e_tiles, vaug_tiles, ctxTn):
            """PV matmul with ones-col -> d; normalize; evict into ctxTn."""
            ps = pcx.tile([128, T], F32, tag="ctx", bufs=2)
            nkt = len(e_tiles)
            for kt in range(nkt):
                nc.tensor.matmul(ps[0:65, :],
                                 vaug_tiles[kt][:, h * 65:(h + 1) * 65],
                                 e_tiles[kt], start=(kt == 0),
                                 stop=(kt == nkt - 1))
            rec = act.tile([1, T], F32, tag="rec", bufs=4)
            nc.vector.reciprocal(out=rec, in_=ps[64:65, :])
            sc = drp.tile([1, T], F32, tag="dsc", bufs=8)
            nc.sync.dma_start(out=sc, in_=rec)
            dbc = act.tile([64, T], F32, tag="dbc", bufs=4)
            nc.sync.dma_start(out=dbc, in_=_dbc_ap(sc, 64, T))
            pi = (h % 2) * 64
            nc.vector.tensor_tensor(out=ctxTn[h // 2][pi:pi + 64, :],
                                    in0=ps[0:64, :], in1=dbc, op=ALU.mult)

        def bcast_row(row_ap, nparts):
            sc = drp.tile([1, T], F32, tag="dsc", bufs=8)
            nc.sync.dma_start(out=sc, in_=row_ap)
            t = act.tile([nparts, T], F32, tag=f"bc{nparts}", bufs=3)
            nc.sync.dma_start(out=t, in_=_dbc_ap(sc, nparts, T))
            return t

        def layer_norm_T(s_tiles, out_tag, gb, dests=None):
            """LN over the partition (H) axis of transposed [H,T] activation."""
            psmu = psc.tile([128, T], F32, tag="sc", bufs=3)
            for k in range(KT):
                nc.tensor.matmul(psmu[0:1, :], ones128, s_tiles[k],
                                 start=(k == 0), stop=(k == KT - 1))
            pss2 = psc.tile([128, T], F32, tag="sc", bufs=3)
            for k in range(KT):
                sq = act.tile([128, T], F32R, tag="sq", bufs=4)
                nc.scalar.activation(sq, s_tiles[k], ACTF.Square)
                nc.tensor.matmul(pss2[0:1, :], ones128, sq,
                                 start=(k == 0), stop=(k == KT - 1))
            def stat():
                return act.tile([1, T], F32, tag="lnstat", bufs=6,
                                name="lnstat")
            mu, ex2, var, rstd, nm = (stat() for _ in range(5))
            nc.scalar.activation(mu, psmu[0:1, :], ACTF.Copy, scale=1.0 / H)
            nc.scalar.activation(ex2, pss2[0:1, :], ACTF.Copy, scale=1.0 / H)
            nc.vector.scalar_tensor_tensor(var, mu, -1.0, mu,
                                           op0=ALU.mult, op1=ALU.mult)
            nc.vector.tensor_tensor(out=var, in0=ex2, in1=var, op=ALU.add)
            nc.scalar.activation(var, var, ACTF.Sqrt, bias=eps_t)
            nc.vector.reciprocal(rstd, var)
            nc.vector.scalar_tensor_tensor(nm, mu, -1.0, rstd,
                                           op0=ALU.mult, op1=ALU.mult)
            RS = bcast_row(rstd, 128)
            NM = bcast_row(nm, 128)
            res = []
            for k in range(KT):
                o = (dests[k] if dests is not None else
                     act.tile([128, T], F32R, tag=out_tag, bufs=4, name="lno"))
                nc.vector.tensor_tensor(out=o, in0=s_tiles[k], in1=RS,
                                        op=ALU.mult)
                nc.vector.tensor_tensor(out=o, in0=o, in1=NM, op=ALU.add)
                if gb is not None:
                    nc.vector.tensor_scalar(o, o, gb[0][k], gb[1][k],
                                            op0=ALU.mult, op1=ALU.add)
                res.append(o)
            return res

        # ---------------- Stage T: tendency attention + LN1 ----------------
        wq_t = [wts.tile([128, H], F32R, tag="w", bufs=16, name="twq") for _ in range(KT)]
        wo_t = [wts.tile([128, H], F32R, tag="w", bufs=16, name="two") for _ in range(KT)]
        for k in range(KT):
            nc.sync.dma_start(out=wq_t[k], in_=tWq[k * 128:(k + 1) * 128, :])
            nc.sync.dma_start(out=wo_t[k], in_=tWo[k * 128:(k + 1) * 128, :])

        for b in range(BL):
            xTb = []
            for k in range(KT):
                t = act.tile([128, T], F32R, tag="xT", bufs=6)
                nc.sync.dma_start(out=t, in_=xT[b, k * 128:(k + 1) * 128, :])
                xTb.append(t)
            qT = proj_T(wq_t, xTb, tbq_t, "qT")
            ctxTn = [act.tile([128, T], F32R, tag="ctxTn", bufs=8,
                              name="ctxTn") for _ in range(KT)]
            for h in range(NH):
                pi = (h % 2) * 64
                sps = psc.tile([128, T], F32, tag="sc", bufs=3)
                nc.tensor.matmul(sps[0:E, :], kTe[h // 2][pi:pi + 64, :],
                                 qT[h // 2][pi:pi + 64, :],
                                 start=True, stop=True)
                et = act.tile([E, T], F32R, tag="et", bufs=4)
                nc.scalar.activation(et, sps[0:E, :], ACTF.Exp)
                softmax_pv(h, [et], [vea], ctxTn)
            s1 = []
            for mo in range(KT):
                ps = pmm.tile([128, T], F32, tag="mm", bufs=3)
                for ki in range(KT):
                    nc.tensor.matmul(ps, wo_t[ki][:, mo * 128:(mo + 1) * 128],
                                     ctxTn[ki], start=(ki == 0),
                                     stop=(ki == KT - 1))
                s = act.tile([128, T], F32R, tag="s1", bufs=8)
                nc.vector.scalar_tensor_tensor(s, ps, tbo_t[mo], xTb[mo],
                                               op0=ALU.add, op1=ALU.add)
                s1.append(s)
            layer_norm_T(s1, "httmp",
                         (g1_t, b1_t) if apply_g1b1 else None, dests=htT[b])

        # ---------------- Branch stages ----------------
        for i in range(4):
            wq_b = [wts.tile([128, H], F32R, tag="w", bufs=16, name="bwq")
                    for _ in range(KT)]
            wk_b = [wts.tile([128, H], F32R, tag="w", bufs=16, name="bwk")
                    for _ in range(KT)]
            wv_b = [wts.tile([128, H], F32R, tag="w", bufs=16, name="bwv")
                    for _ in range(KT)]
            wh_b = [wts.tile([128, H], F32R, tag="w", bufs=16, name="bwh")
                    for _ in range(KT)]
            for k in range(KT):
                sl = slice(k * 128, (k + 1) * 128)
                nc.sync.dma_start(out=wq_b[k], in_=bWq[i, sl, :])
                nc.sync.dma_start(out=wk_b[k], in_=bWk[i, sl, :])
                nc.sync.dma_start(out=wv_b[k], in_=bWv[i, sl, :])
                nc.sync.dma_start(out=wh_b[k], in_=What[i, sl, :])
            for b in range(BL):
                fl = fr = None
                if i != 1:
                    fl = act.tile([3, T], F32R, tag="fl", bufs=2)
                    nc.sync.dma_start(out=fl, in_=flq[b])
                    if i == 0:
                        fr = act.tile([1, T], F32R, tag="frg", bufs=2)
                        nc.sync.dma_start(out=fr, in_=frg[b])
                    else:
                        fr = act.tile([3, T], F32R, tag="fr", bufs=2)
                        nc.sync.dma_start(
                            out=fr, in_=(frintra[b] if i == 2 else frinter[b]))
                ml = None
                if i == 1:  # local: multiplicative mask  band*outer(col,row)+B
                    lv = []
                    for rix in range(3):
                        lt = act.tile([1, T], F32R, tag=f"lv{rix}", bufs=2)
                        nc.sync.dma_start(out=lt, in_=locv[b, rix])
                        lv.append(lt)
                    ml = []
                    for m in range(MT):
                        msl = slice(m * 128, (m + 1) * 128)
                        psA = psc.tile([128, T], F32, tag="sc", bufs=3)
                        nc.tensor.matmul(psA, lv[0][:, msl], lv[1],
                                         start=True, stop=True)
                        psB = psc.tile([128, T], F32, tag="sc", bufs=3)
                        nc.tensor.matmul(psB, onr[:, msl], lv[2],
                                         start=True, stop=True)
                        mt_ = act.tile([128, T], F32, tag="ml", bufs=4)
                        nc.vector.tensor_tensor(out=mt_, in0=psA,
                                                in1=bandt[m], op=ALU.mult)
                        nc.vector.tensor_tensor(out=mt_, in0=mt_, in1=psB,
                                                op=ALU.add)
                        ml.append(mt_)
                qT = proj_T(wq_b, htT[b], bbq_t[i], "qT")
                kT = proj_T(wk_b, htT[b], bbk_t[i], "kT")
                va = []
                for mo in range(MT):
                    ps = pmm.tile([128, H], F32, tag="mm", bufs=3)
                    for ki in range(KT):
                        nc.tensor.matmul(
                            ps, htT[b][ki][:, mo * 128:(mo + 1) * 128],
                            wv_b[ki], start=(ki == 0), stop=(ki == KT - 1))
                    vt = act.tile([128, NH * 65], F32R, tag="va", bufs=4)
                    nc.vector.tensor_copy(
                        out=vt.rearrange("p (h d) -> p h d", h=NH)[:, :, 0:64],
                        in_=ps.rearrange("p (h d) -> p h d", h=NH))
                    nc.sync.dma_start(
                        out=vt.rearrange("p (h d) -> p h d", h=NH)[:, :, 64:65],
                        in_=bass.AP(tensor=onesd.tensor, offset=onesd.offset,
                                    ap=[[0, 128], [1, NH]]))
                    va.append(vt)
                ctxTn = [act.tile([128, T], F32R, tag="ctxTn", bufs=8,
                                  name="ctxTn") for _ in range(KT)]
                for h in range(NH):
                    pi = (h % 2) * 64
                    ets = []
                    for m in range(MT):
                        msl = slice(m * 128, (m + 1) * 128)
                        sps = psc.tile([128, T], F32, tag="sc", bufs=3)
                        nc.tensor.matmul(sps, kT[h // 2][pi:pi + 64, msl],
                                         qT[h // 2][pi:pi + 64, :],
                                         start=True, stop=(i == 1))
                        if i == 0:
                            nc.tensor.matmul(sps, fr[:, msl], fl[0:1, :],
                                             start=False, stop=True)
                        elif i in (2, 3):
                            nc.tensor.matmul(sps, fr[:, msl], fl,
                                             start=False, stop=True)
                        et = act.tile([128, T], F32R, tag="et", bufs=4)
                        nc.scalar.activation(et, sps, ACTF.Exp)
                        if i == 1:
                            nc.vector.tensor_tensor(out=et, in0=et, in1=ml[m],
                                                    op=ALU.mult)
                        ets.append(et)
                    softmax_pv(h, ets, va, ctxTn)
                for mo in range(KT):
                    ps = pmm.tile([128, T], F32, tag="mm", bufs=3)
                    for ki in range(KT):
                        nc.tensor.matmul(
                            ps, wh_b[ki][:, mo * 128:(mo + 1) * 128],
                            ctxTn[ki], start=(ki == 0), stop=(ki == KT - 1))
                    if i == 0:
                        nc.scalar.copy(out=h2sb[b][mo], in_=ps)
                    elif i < 3:
                        nc.vector.tensor_tensor(out=h2sb[b][mo],
                                                in0=h2sb[b][mo], in1=ps,
                                                op=ALU.add)
                    else:
                        nc.vector.scalar_tensor_tensor(
                            h2sb[b][mo], ps, bhat_t[mo], h2sb[b][mo],
                            op0=ALU.add, op1=ALU.add)

        # ---------------- Final: residual + LN2 + W2 ----------------
        w2_t = [wts.tile([128, H], F32R, tag="w", bufs=16, name="w2p") for _ in range(KT)]
        for k in range(KT):
            nc.sync.dma_start(out=w2_t[k], in_=W2p[k * 128:(k + 1) * 128, :])
        for b in range(BL):
            for k in range(KT):
                nc.vector.tensor_tensor(out=h2sb[b][k], in0=h2sb[b][k],
                                        in1=htT[b][k], op=ALU.add)
            n2 = layer_norm_T(h2sb[b], "n2", None)
            for mo in range(MT):
                ps = pmm.tile([128, H], F32, tag="mm", bufs=3)
                for ki in range(KT):
                    nc.tensor.matmul(
                        ps, n2[ki][:, mo * 128:(mo + 1) * 128],
                        w2_t[ki], start=(ki == 0), stop=(ki == KT - 1))
                osb = act.tile([128, H], F32, tag="osb", bufs=2)
                if apply_c2:
                    nc.vector.tensor_tensor(out=osb, in0=ps, in1=c2n,
                                            op=ALU.add)
                else:
                    nc.scalar.copy(out=osb, in_=ps)
                nc.sync.dma_start(out=out[b, mo * 128:(mo + 1) * 128, :],
                                  in_=osb)
        drp.release()
        pcx.release()
        psc.release()
        pmm.release()
        act.release()
        wts.release()
        per.release()
        cst.release()

    nc.compile()
    return nc


def _host_prep(inputs):
    f32 = np.float32
    g = {}
    x = np.asarray(inputs["x"], f32)
    lengths = np.asarray(inputs["lengths"])
    speakers = np.asarray(inputs["speakers"])
    emo = np.asarray(inputs["emo_table"], f32)

    per = {}
    per["xT"] = np.ascontiguousarray(x.transpose(0, 2, 1))
    j = np.arange(T)
    row = (j[None, :] < lengths[:, None]).astype(f32)          # [B,T]
    col = row
    sp = speakers.astype(f32)
    u1 = row * sp
    u2 = row * (1.0 - sp)
    ones = np.ones_like(row)
    per["flq"] = np.ascontiguousarray(
        np.stack([ones, u1, u2], axis=1))
    per["frintra"] = np.ascontiguousarray(
        np.stack([NEG * ones, -NEG * sp, -NEG * (1.0 - sp)], axis=1))
    per["frinter"] = np.ascontiguousarray(
        np.stack([NEG * ones, -NEG * (1.0 - sp) * col, -NEG * sp * col],
                 axis=1))
    per["frg"] = np.ascontiguousarray((NEG * (1.0 - col))[:, None, :])
    per["locv"] = np.ascontiguousarray(
        np.stack([col, row, 1.0 - row], axis=1))

    band = (np.abs(j[:, None] - j[None, :]) <= 2).astype(f32)
    g["bandd"] = band
    kemo = (emo @ np.asarray(inputs["t_Wk"], f32)
            + np.asarray(inputs["t_bk"], f32))
    g["kTemo"] = np.ascontiguousarray(kemo.T)
    vemo = (emo @ np.asarray(inputs["t_Wv"], f32)
            + np.asarray(inputs["t_bv"], f32))
    vaug = np.zeros((E, NH * 65), f32)
    for h in range(NH):
        vaug[:, h * 65:h * 65 + 64] = vemo[:, h * 64:(h + 1) * 64]
        vaug[:, h * 65 + 64] = 1.0
    g["vemoaug"] = vaug
    g["tWq"] = np.asarray(inputs["t_Wq"], f32) / np.sqrt(DH).astype(f32)
    g["tWo"] = np.asarray(inputs["t_Wo"], f32)
    g["bWq"] = np.asarray(inputs["b_Wq"], f32) / np.sqrt(DH).astype(f32)
    g["bWk"] = np.asarray(inputs["b_Wk"], f32)
    g["bWv"] = np.asarray(inputs["b_Wv"], f32)
    W1 = np.asarray(inputs["W1"], np.float64)
    bWo = np.asarray(inputs["b_Wo"], np.float64)
    g["What"] = np.stack(
        [(bWo[i] @ W1[i * H:(i + 1) * H]).astype(f32) for i in range(4)])
    ln2g = np.asarray(inputs["ln2_g"], np.float64)
    g["W2p"] = (ln2g[:, None] * np.asarray(inputs["W2"], np.float64)).astype(f32)
    g["onesd"] = np.ones(T, f32)
    g["tbq"] = np.asarray(inputs["t_bq"], f32) / np.sqrt(DH).astype(f32)
    g["tbo"] = (np.asarray(inputs["t_bo"], np.float64)
                + np.asarray(inputs["t_bv"], np.float64)
                @ np.asarray(inputs["t_Wo"], np.float64)).astype(f32)
    g["g1"] = np.asarray(inputs["t_ln_g"], f32)
    g["b1v"] = np.asarray(inputs["t_ln_b"], f32)
    g["bbq"] = (np.asarray(inputs["b_bq"], f32)
                / np.sqrt(DH).astype(f32))
    g["bbk"] = np.asarray(inputs["b_bk"], f32)
    bhat = np.asarray(inputs["b1"], np.float64).copy()
    for i in range(4):
        eff = (np.asarray(inputs["b_bo"][i], np.float64)
               + np.asarray(inputs["b_bv"][i], np.float64) @ bWo[i])
        bhat += eff @ W1[i * H:(i + 1) * H]
    g["bhat"] = bhat.astype(f32)
    g["c2row"] = (np.asarray(inputs["ln2_b"], np.float64)
                  @ np.asarray(inputs["W2"], np.float64)).astype(f32)

    apply_g1b1 = not (np.all(inputs["t_ln_g"] == 1.0)
                      and np.all(inputs["t_ln_b"] == 0.0))
    apply_c2 = bool(np.any(g["c2row"] != 0.0))

    in_maps = []
    for c in range(NCORES):
        m = dict(g)
        for k, v in per.items():
            m[k] = np.ascontiguousarray(v[c * BL:(c + 1) * BL])
        in_maps.append(m)
    return in_maps, apply_g1b1, apply_c2


def kernel(**inputs):
    in_maps, apply_g1b1, apply_c2 = _host_prep(inputs)
    key = (apply_g1b1, apply_c2)
    if key not in _CACHE:
        _CACHE[key] = _build(*key)
    nc = _CACHE[key]
    res = run_bass_kernel_spmd(nc, in_maps, core_ids=list(range(NCORES)),
                               trace=False)
    outs = [res.results[c]["out"] for c in range(NCORES)]
    return np.concatenate(outs, axis=0)
